# revision 1
# baseline (speedup 1.0000x reference)
"""AGMBrain GNN message passing — data-parallel across 8 trn2 NeuronCores.

Sharding: batch dim of x split 8 ways (2048 rows/core), all parameters
replicated (edge_vectors is only 512 KB). No cross-device communication.
Each core runs the full pipeline (input proj -> 3 message-passing steps ->
recreation/score heads) on its batch shard; host concatenates shard outputs.
"""

import numpy as np
import jax
import jax.numpy as jnp

N_STEPS = 3


def _model(x, W_in, b_in, neuron_states, edge_vectors, W_rec, b_rec, W_score, b_score):
    x_t = x @ W_in + b_in                                  # (b, d)
    states = neuron_states[None, :, :] + x_t[:, None, :]   # (b, n, d)
    n = neuron_states.shape[0]
    off_diag = (1.0 - jnp.eye(n, dtype=states.dtype))
    for _ in range(N_STEPS):
        scores = jnp.einsum('ijq,biq->bij', edge_vectors, states) * off_diag
        states = jax.nn.relu(jnp.einsum('bij,bjp->bip', scores, states))
    out = states[:, -1]
    recreation = out @ W_rec + b_rec
    score_out = out @ W_score + b_score
    return recreation, score_out


def kernel(x, W_in, b_in, neuron_states, edge_vectors, W_rec, b_rec, W_score, b_score):
    params = (W_in, b_in, neuron_states, edge_vectors, W_rec, b_rec, W_score, b_score)
    x = np.asarray(x)
    B = x.shape[0]
    try:
        devs = jax.devices()
        nd = 8 if len(devs) >= 8 else len(devs)
        if nd > 1 and B % nd == 0:
            xs = x.reshape(nd, B // nd, x.shape[1])
            pf = jax.pmap(_model, in_axes=(0,) + (None,) * 8, devices=devs[:nd])
            rec, sc = pf(xs, *params)
            rec = np.asarray(rec, dtype=np.float32).reshape(B, -1)
            sc = np.asarray(sc, dtype=np.float32).reshape(B, -1)
            return rec, sc
    except Exception:
        pass
    rec, sc = jax.jit(_model)(x, *params)
    return np.asarray(rec, np.float32), np.asarray(sc, np.float32)


if __name__ == "__main__":
    rng = np.random.default_rng(0)
    x = rng.standard_normal((16384, 256), dtype=np.float32)
    W_in = rng.standard_normal((256, 128), dtype=np.float32) / 16.0
    b_in = rng.standard_normal((128,), dtype=np.float32) * 0.01
    ns = rng.standard_normal((32, 128), dtype=np.float32)
    ev = rng.standard_normal((32, 32, 128), dtype=np.float32)
    W_rec = rng.standard_normal((128, 256), dtype=np.float32) / 11.3
    b_rec = rng.standard_normal((256,), dtype=np.float32) * 0.01
    W_sc = rng.standard_normal((128, 1), dtype=np.float32) / 11.3
    b_sc = rng.standard_normal((1,), dtype=np.float32) * 0.01
    r, s = kernel(x, W_in, b_in, ns, ev, W_rec, b_rec, W_sc, b_sc)
    print(r.shape, s.shape, r.dtype, s.dtype)
